# revision 1
# baseline (speedup 1.0000x reference)
"""Trainium2 Bass kernel for nn_MMHA_78039555768536.

Gated mix of per-segment causal softmax attention and a linear-attention
memory (delta rule, memory summed over batch per segment).

Strategy (8 cores): reformulate the memory recurrence as a linear matrix
recurrence  M_{t+1} = A_t M_t + B_t  with
    A_t = I - sum_b sk_b^T diag(1/d_b) sk_b   (symmetric A-part)
    B_t = sum_b sk_b^T v_b
    d_b = sk_b @ z_{b,t};  z is a prefix of column-sums of sk (M-independent)
Core c owns segments {2c, 2c+1} for all batches.  Two all-gathers:
 AG1: per-segment colsums of sk (for the z prefix)  [tiny]
 AG2: per-core pair composition (Abar^T, Bbar)      [1 MB bf16 per rank]
Then every core redundantly runs the 7-step pair chain and selects its own
prefix M via a per-core one-hot input (SPMD, no branches).

All matmul operands are bf16 (fp32 PSUM accumulation); validated vs the
fp32 reference at ~3e-3 relative-to-absmax error in a numpy prototype.
"""

import os
import sys

sys.path.insert(0, "/opt/trn_rl_repo")

STAGE = int(os.environ.get("KSTAGE", "9"))
SUB = int(os.environ.get("KSUB", "9"))

from contextlib import ExitStack

import numpy as np
import ml_dtypes

import concourse.bass as bass
import concourse.bacc as bacc
import concourse.tile as tile
from concourse import mybir
from concourse import bass_utils

B, L, DIN = 4, 8192, 512
H, D, SEG = 8, 64, 512
HD = H * D
NSEG = L // SEG          # 16
NC = 8                   # cores
SPC = NSEG // NC         # segments per core = 2
P = 128
NB = HD // P             # 4 blocks of 128
BS = B * SPC             # batch-segment units per core = 8

bf = mybir.dt.bfloat16
f32 = mybir.dt.float32
AF = mybir.ActivationFunctionType
OP = mybir.AluOpType
bf_np = ml_dtypes.bfloat16

_CACHE = {}


def _build():
    nc = bacc.Bacc(
        "TRN2",
        target_bir_lowering=False,
        debug=False,
        enable_asserts=False,
        num_devices=NC,
    )

    # ---------------- DRAM I/O ----------------
    xt_d = nc.dram_tensor("xt", [B, SPC, NB, P, SEG], bf, kind="ExternalInput").ap()
    wq_d = nc.dram_tensor("wq", [NB, P, HD], bf, kind="ExternalInput").ap()
    wk_d = nc.dram_tensor("wk", [NB, P, HD], bf, kind="ExternalInput").ap()
    wv_d = nc.dram_tensor("wv", [NB, P, HD], bf, kind="ExternalInput").ap()
    wd_d = nc.dram_tensor("wd", [NB, P, D], bf, kind="ExternalInput").ap()
    gcol_d = nc.dram_tensor("gcol", [P, NB], f32, kind="ExternalInput").ap()
    omg_d = nc.dram_tensor("omg", [P, NB], f32, kind="ExternalInput").ap()
    zmask_d = nc.dram_tensor("zmask", [64, NC], f32, kind="ExternalInput").ap()
    oh_d = nc.dram_tensor("oh", [P, NC], f32, kind="ExternalInput").ap()
    mask_d = nc.dram_tensor("cmask", [P, P], bf, kind="ExternalInput").ap()
    ident_d = nc.dram_tensor("ident", [P, P], bf, kind="ExternalInput").ap()
    out_d = nc.dram_tensor("out", [B, SPC, SEG, D], f32, kind="ExternalOutput").ap()

    with tile.TileContext(nc) as tc, ExitStack() as ctx:
        # ---------------- constant / DRAM pools ----------------
        const = ctx.enter_context(tc.tile_pool(name="const", bufs=1))
        dram = ctx.enter_context(tc.tile_pool(name="dram", bufs=1, space="DRAM"))
        keep = ctx.enter_context(tc.tile_pool(name="keep", bufs=BS))
        phb = ctx.enter_context(tc.tile_pool(name="phb", bufs=1))  # phase-B singles

        WQ = const.tile([P, NB, HD], bf)
        WK = const.tile([P, NB, HD], bf)
        WV = const.tile([P, NB, HD], bf)
        WD = const.tile([P, NB, D], bf)
        GC = const.tile([P, NB], f32)
        OMG = const.tile([P, NB], f32)
        ZM = const.tile([64, NC], f32)
        OH = const.tile([P, NC], f32)
        CM = const.tile([P, P], bf)
        ID = const.tile([P, P], bf)
        ONE = const.tile([P, 1], bf)

        nc.sync.dma_start(out=WQ, in_=wq_d.rearrange("kb p n -> p kb n"))
        nc.sync.dma_start(out=WK, in_=wk_d.rearrange("kb p n -> p kb n"))
        nc.sync.dma_start(out=WV, in_=wv_d.rearrange("kb p n -> p kb n"))
        nc.sync.dma_start(out=WD, in_=wd_d.rearrange("kb p n -> p kb n"))
        nc.sync.dma_start(out=GC, in_=gcol_d)
        nc.sync.dma_start(out=OMG, in_=omg_d)
        nc.sync.dma_start(out=ZM, in_=zmask_d)
        nc.sync.dma_start(out=OH, in_=oh_d)
        nc.sync.dma_start(out=CM, in_=mask_d)
        nc.sync.dma_start(out=ID, in_=ident_d)
        nc.vector.memset(ONE, 1.0)

        # collective bounce buffers
        cs_in = dram.tile([BS, HD], f32)
        cs_out = dram.tile([NC * BS, HD], f32)
        ab_in = dram.tile([2, HD, HD], bf)
        zrow_d = dram.tile([BS, HD], bf)
        rca_d = dram.tile([BS, H, SEG], bf)
        rcm_d = dram.tile([BS, SEG], bf)
        ab_out = dram.tile([NC, 2, HD, HD], bf)

        # retained across phases (bufs=BS -> one slot per batch-segment)
        skT = [keep.tile([P, NB, HD], bf, tag="sk", name=f"sk{i}") for i in range(BS)]
        sqT = [keep.tile([P, NB, SEG], bf, tag="sq", name=f"sq{i}") for i in range(BS)]
        step_d = dram.tile([BS, NB, P, SEG], bf)  # attention-term scratch

        # z tiles (phase boundary singles)
        ZROW = phb.tile([BS, HD], f32)      # z at segment start, row form
        ZCOL = phb.tile([P, NB, BS], bf)    # column form for denominators
        AT0 = phb.tile([P, NB, HD], bf)     # segment-0 A-part (retained)
        BT0 = phb.tile([P, NB, HD], bf)
        MSEL = phb.tile([P, NB, HD], bf)    # selected M at segment 2c
        MLOC1 = phb.tile([P, NB, HD], bf)   # M at segment 2c+1

        def bs_of(b, j):
            return j * B + b

        # ============ PHASE A1: k-projection, sk, colsums ============
        with tc.tile_pool(name="pa1", bufs=2) as pa1, \
             tc.tile_pool(name="ps1", bufs=2, space="PSUM") as ps1:
            for j in range(SPC):
                for b in range(B):
                    i = bs_of(b, j)
                    XT = pa1.tile([P, NB, SEG], bf, tag="xt")
                    nc.sync.dma_start(out=XT, in_=xt_d[b, j].rearrange("kb p s -> p kb s"))
                    sk_i = skT[i]
                    for sb in range(NB):
                        pk = ps1.tile([P, SEG], f32, tag="pk")
                        for kb in range(NB):
                            nc.tensor.matmul(
                                pk,
                                lhsT=XT[:, kb, sb * P:(sb + 1) * P],
                                rhs=WK[:, kb, :],
                                start=(kb == 0),
                                stop=(kb == NB - 1),
                            )
                        # elu1(k) = max(k + 1, exp(min(k, 0)))
                        em = pa1.tile([P, SEG], bf, tag="em")
                        nc.vector.tensor_scalar_min(em, pk, 0.0)
                        ee = pa1.tile([P, SEG], bf, tag="ee")
                        nc.scalar.activation(ee, em, AF.Exp)
                        nc.vector.scalar_tensor_tensor(
                            out=sk_i[:, sb, :], in0=pk, scalar=1.0, in1=ee,
                            op0=OP.add, op1=OP.max,
                        )
                    pc = ps1.tile([1, HD], f32, tag="pc")
                    for sb in range(NB):
                        nc.tensor.matmul(
                            pc, lhsT=ONE, rhs=sk_i[:, sb, :],
                            start=(sb == 0), stop=(sb == NB - 1),
                        )
                    cs_sb = pa1.tile([1, HD], f32, tag="cs")
                    nc.scalar.activation(cs_sb, pc, AF.Copy)
                    nc.sync.dma_start(out=cs_in[i:i + 1, :], in_=cs_sb)

        if STAGE >= 2:
            # ============ AG1: colsums ============
            nc.gpsimd.collective_compute(
                "AllGather", OP.bypass,
                replica_groups=[list(range(NC))],
                ins=[cs_in.opt()], outs=[cs_out.opt()],
            )

            # ============ z prefix ============
            with tc.tile_pool(name="pz", bufs=1) as pz, \
                 tc.tile_pool(name="psz", bufs=1, space="PSUM") as psz:
                Z = pz.tile([NC * BS, HD], f32, tag="z")
                nc.sync.dma_start(out=Z, in_=cs_out)
                zp = psz.tile([BS, HD], f32, tag="zp")
                nc.tensor.matmul(zp, lhsT=ZM, rhs=Z, start=True, stop=True)
                nc.scalar.activation(ZROW, zp, AF.Copy, bias=1.0 / D)
                ZROW16 = pz.tile([BS, HD], bf, tag="zr16")
                nc.vector.tensor_copy(ZROW16, ZROW)
                nc.sync.dma_start(out=zrow_d, in_=ZROW16)
                for kb in range(NB):
                    zc = psz.tile([P, BS], f32, tag="zc")
                    nc.tensor.matmul(zc, lhsT=Z[:, kb * P:(kb + 1) * P], rhs=ZM,
                                     start=True, stop=True)
                    nc.scalar.activation(ZCOL[:, kb, :], zc, AF.Copy, bias=1.0 / D)


        if STAGE >= 3:
            # ============ PHASE A2: per-segment q/kT/v proj, attention, d/skd, A/B ============
            with tc.tile_pool(name="pa2", bufs=2) as pa2, \
                 tc.tile_pool(name="pskd", bufs=B) as pskd, \
                 tc.tile_pool(name="pva", bufs=B) as pva, \
                 tc.tile_pool(name="pw", bufs=3) as pw, \
                 tc.tile_pool(name="pab", bufs=1) as pab, \
                 tc.tile_pool(name="ps2", bufs=2, space="PSUM") as ps2, \
                 tc.tile_pool(name="psc", bufs=2, space="PSUM") as psc, \
                 tc.tile_pool(name="psa", bufs=2, space="PSUM") as psa:
                at1 = bt1 = None
                for j in range(SPC):
                    skd = [None] * B
                    vaug = [None] * B
                    for b in range(B):
                        i = bs_of(b, j)
                        XT = pa2.tile([P, NB, SEG], bf, tag="xt")
                        nc.sync.dma_start(out=XT, in_=xt_d[b, j].rearrange("kb p s -> p kb s"))

                        # --- qT (transposed: hd on partitions) ---
                        qh = pa2.tile([P, NB, SEG], bf, tag="qh")
                        sq_i = sqT[i]
                        for mb in range(NB):
                            pq = ps2.tile([P, SEG], f32, tag="pp")
                            for kb in range(NB):
                                nc.tensor.matmul(
                                    pq, lhsT=WQ[:, kb, mb * P:(mb + 1) * P],
                                    rhs=XT[:, kb, :],
                                    start=(kb == 0), stop=(kb == NB - 1),
                                )
                            nc.scalar.activation(qh[:, mb, :], pq, AF.Copy)
                            em = pa2.tile([P, SEG], bf, tag="em")
                            nc.vector.tensor_scalar_min(em, pq, 0.0)
                            ee = pa2.tile([P, SEG], bf, tag="ee")
                            nc.scalar.activation(ee, em, AF.Exp)
                            nc.vector.scalar_tensor_tensor(
                                out=sq_i[:, mb, :], in0=pq, scalar=1.0, in1=ee,
                                op0=OP.add, op1=OP.max,
                            )
                        # --- kT ---
                        kh = pa2.tile([P, NB, SEG], bf, tag="kh", bufs=1)
                        for mb in range(NB):
                            pkt = ps2.tile([P, SEG], f32, tag="pp")
                            for kb in range(NB):
                                nc.tensor.matmul(
                                    pkt, lhsT=WK[:, kb, mb * P:(mb + 1) * P],
                                    rhs=XT[:, kb, :],
                                    start=(kb == 0), stop=(kb == NB - 1),
                                )
                            nc.scalar.activation(kh[:, mb, :], pkt, AF.Copy)
                        # --- v (original orientation) + aug ones column ---
                        va = pva.tile([P, NB, H, D + 1], bf, tag="va")
                        vaug[b] = va
                        nc.vector.memset(va[:, :, :, D:D + 1], 1.0)
                        for sb in range(NB):
                            pv = ps2.tile([P, SEG], f32, tag="pp")
                            for kb in range(NB):
                                nc.tensor.matmul(
                                    pv, lhsT=XT[:, kb, sb * P:(sb + 1) * P],
                                    rhs=WV[:, kb, :],
                                    start=(kb == 0), stop=(kb == NB - 1),
                                )
                            nc.vector.tensor_copy(
                                va[:, sb, :, 0:D], pv.rearrange("p (h d) -> p h d", h=H)
                            )

                        if SUB >= 2:
                            # --- attention ---
                            st_i = pa2.tile([P, NB, SEG], bf, tag="stp", name=f"stp{i}")
                            for h in range(H):
                                hb, ho = h // 2, (h % 2) * 64
                                pat = psa.tile([D + 1, SEG], f32, tag="at")
                                for kb in range(NB):
                                    q0 = kb * P
                                    qf = SEG - q0
                                    ps_ = psc.tile([P, SEG], f32, tag="sc")
                                    nc.tensor.matmul(
                                        ps_[:, 0:qf],
                                        lhsT=kh[ho:ho + 64, hb, q0:q0 + P],
                                        rhs=qh[ho:ho + 64, hb, q0:SEG],
                                        start=True, stop=True,
                                    )
                                    wt = pw.tile([P, SEG], bf, tag="wt")
                                    nc.scalar.activation(wt[:, 0:qf], ps_[:, 0:qf], AF.Exp,
                                                         scale=0.125)
                                    # causal mask on the diagonal 128x128 block
                                    nc.vector.tensor_mul(wt[:, 0:P], wt[:, 0:P], CM)
                                    nc.tensor.matmul(
                                        pat[:, q0:SEG],
                                        lhsT=va[:, kb, h, :],
                                        rhs=wt[:, 0:qf],
                                        start=(kb == 0), stop=(kb == NB - 1),
                                    )
                                rca = pw.tile([1, SEG], bf, tag="rca")
                                with nc.allow_low_precision(reason="bf16 softmax recip"):
                                    nc.vector.reciprocal(rca, pat[D:D + 1, :])
                                nc.sync.dma_start(out=rca_d[i, h], in_=rca)
                                rcab = pw.tile([P, SEG], bf, tag="rcab")
                                nc.gpsimd.dma_start(
                                    out=rcab[ho:ho + D, :],
                                    in_=rca_d[i:i + 1, h, :].partition_broadcast(D))
                                nc.vector.scalar_tensor_tensor(
                                    out=st_i[ho:ho + 64, hb, :],
                                    in0=pat[0:D, :],
                                    scalar=OMG[ho:ho + 64, hb:hb + 1],
                                    in1=rcab[ho:ho + D, :],
                                    op0=OP.mult, op1=OP.mult,
                                )

                            nc.sync.dma_start(
                                out=step_d[i].rearrange("kb p s -> p kb s"), in_=st_i)


                        if SUB >= 3:
                            # --- d and sk/d ---
                            i_row = bs_of(b, j)
                            sk_i = skT[i]
                            sd = pskd.tile([P, NB, HD], bf, tag="skd")
                            skd[b] = sd
                            dcol = pa2.tile([P, NB], f32, tag="d")
                            rcd = pa2.tile([P, NB], f32, tag="rcd")
                            jnk = pa2.tile([P, HD], bf, tag="jnk", bufs=1)
                            zbp = pa2.tile([P, HD], bf, tag="zbp")
                            nc.gpsimd.dma_start(
                                out=zbp,
                                in_=zrow_d[i_row:i_row + 1, :].partition_broadcast(P))
                            for sb in range(NB):
                                nc.vector.tensor_mul(jnk, sk_i[:, sb, :], zbp)
                                nc.vector.tensor_reduce(
                                    out=dcol[:, sb:sb + 1], in_=jnk,
                                    axis=mybir.AxisListType.X, op=OP.add,
                                )
                            nc.vector.reciprocal(rcd, dcol)
                            for sb in range(NB):
                                nc.vector.tensor_scalar_mul(
                                    sd[:, sb, :], sk_i[:, sb, :], rcd[:, sb:sb + 1]
                                )


                    if SUB >= 4:
                        # --- A_t, B_t for this segment (sum over batches) ---
                        at_t = pab.tile([P, NB, HD], bf, tag="at", name=f"at{j}") if j > 0 else AT0
                        bt_t = pab.tile([P, NB, HD], bf, tag="bt", name=f"bt{j}") if j > 0 else BT0
                        for mb in range(NB):
                            pA = ps2.tile([P, HD], f32, tag="pp")
                            n = 0
                            for b in range(B):
                                for sb in range(NB):
                                    nc.tensor.matmul(
                                        pA,
                                        lhsT=skT[bs_of(b, j)][:, sb, mb * P:(mb + 1) * P],
                                        rhs=skd[b][:, sb, :],
                                        start=(n == 0), stop=(n == B * NB - 1),
                                    )
                                    n += 1
                            # negate: A-part = -K
                            nc.scalar.activation(at_t[:, mb, :], pA, AF.Copy, scale=-1.0)
                        for mb in range(NB):
                            pB = ps2.tile([P, HD], f32, tag="pp")
                            n = 0
                            for b in range(B):
                                for sb in range(NB):
                                    nc.tensor.matmul(
                                        pB.rearrange("p (h d) -> p h d", h=H),
                                        lhsT=skT[bs_of(b, j)][:, sb, mb * P:(mb + 1) * P],
                                        rhs=vaug[b][:, sb, :, 0:D],
                                        start=(n == 0), stop=(n == B * NB - 1),
                                    )
                                    n += 1
                            nc.scalar.activation(bt_t[:, mb, :], pB, AF.Copy)
                        if j > 0:
                            at1, bt1 = at_t, bt_t


                if SUB >= 5:
                    # --- pair composition: abA = Abar^T = A0 A1 + A0 + A1 ; abB = Bbar ---
                    abA = pab.tile([P, NB, HD], bf, tag="abA")
                    abB = pab.tile([P, NB, HD], bf, tag="abB")
                    for mb in range(NB):
                        pA = ps2.tile([P, HD], f32, tag="pp")
                        for kb in range(NB):
                            nc.tensor.matmul(
                                pA, lhsT=AT0[:, kb, mb * P:(mb + 1) * P], rhs=at1[:, kb, :],
                                start=(kb == 0), stop=False,
                            )
                        nc.tensor.matmul(pA, lhsT=ID, rhs=AT0[:, mb, :], start=False, stop=False)
                        nc.tensor.matmul(pA, lhsT=ID, rhs=at1[:, mb, :], start=False, stop=True)
                        nc.scalar.activation(abA[:, mb, :], pA, AF.Copy)
                    for mb in range(NB):
                        pB = ps2.tile([P, HD], f32, tag="pp")
                        for kb in range(NB):
                            nc.tensor.matmul(
                                pB, lhsT=at1[:, kb, mb * P:(mb + 1) * P], rhs=BT0[:, kb, :],
                                start=(kb == 0), stop=False,
                            )
                        nc.tensor.matmul(pB, lhsT=ID, rhs=BT0[:, mb, :], start=False, stop=False)
                        nc.tensor.matmul(pB, lhsT=ID, rhs=bt1[:, mb, :], start=False, stop=True)
                        nc.scalar.activation(abB[:, mb, :], pB, AF.Copy)
                    nc.sync.dma_start(out=ab_in[0].rearrange("(kb p) n -> p kb n", p=P), in_=abA)
                    nc.sync.dma_start(out=ab_in[1].rearrange("(kb p) n -> p kb n", p=P), in_=abB)



        if STAGE >= 5:
            # ============ AG2: pair compositions ============
            nc.gpsimd.collective_compute(
                "AllGather", OP.bypass,
                replica_groups=[list(range(NC))],
                ins=[ab_in.opt()], outs=[ab_out.opt()],
            )


        if STAGE >= 6:
            # ============ chain + select ============
            nc.vector.memset(MSEL, 0.0)
            with tc.tile_pool(name="pch", bufs=2) as pch, \
                 tc.tile_pool(name="psch", bufs=NB, space="PSUM") as psch:
                pM = [psch.tile([P, HD], f32, tag="ch", name=f"chain{i}") for i in range(NB)]
                mprev = None
                for step in range(NC - 1):
                    cA = pch.tile([P, NB, HD], bf, tag="cA")
                    cB = pch.tile([P, NB, HD], bf, tag="cB")
                    nc.sync.dma_start(
                        out=cA, in_=ab_out[step, 0].rearrange("(kb p) n -> p kb n", p=P))
                    nc.sync.dma_start(
                        out=cB, in_=ab_out[step, 1].rearrange("(kb p) n -> p kb n", p=P))
                    mcur = pch.tile([P, NB, HD], bf, tag="mc")
                    for mb in range(NB):
                        if step == 0:
                            nc.tensor.matmul(pM[mb], lhsT=ID, rhs=cB[:, mb, :],
                                             start=True, stop=True)
                        else:
                            for kb in range(NB):
                                nc.tensor.matmul(
                                    pM[mb], lhsT=cA[:, kb, mb * P:(mb + 1) * P],
                                    rhs=mprev[:, kb, :],
                                    start=False, stop=False,
                                )
                            nc.tensor.matmul(pM[mb], lhsT=ID, rhs=cB[:, mb, :],
                                             start=False, stop=True)
                        nc.scalar.activation(mcur[:, mb, :], pM[mb], AF.Copy)
                        nc.vector.scalar_tensor_tensor(
                            out=MSEL[:, mb, :], in0=mcur[:, mb, :],
                            scalar=OH[:, step:step + 1], in1=MSEL[:, mb, :],
                            op0=OP.mult, op1=OP.add,
                        )
                    mprev = mcur


        if STAGE >= 7:
            # ============ phase B: M_loc1, mem_ret, combine, Wd ============
            with tc.tile_pool(name="pb", bufs=2) as pb, \
                 tc.tile_pool(name="psb", bufs=2, space="PSUM") as psb, \
                 tc.tile_pool(name="psw", bufs=2, space="PSUM") as psw:
                # M at segment 2c+1 = M + A0-part @ M + B0
                for mb in range(NB):
                    pm = psb.tile([P, HD], f32, tag="mm")
                    for kb in range(NB):
                        nc.tensor.matmul(
                            pm, lhsT=AT0[:, kb, mb * P:(mb + 1) * P], rhs=MSEL[:, kb, :],
                            start=(kb == 0), stop=False,
                        )
                    nc.tensor.matmul(pm, lhsT=ID, rhs=MSEL[:, mb, :], start=False, stop=False)
                    nc.tensor.matmul(pm, lhsT=ID, rhs=BT0[:, mb, :], start=False, stop=True)
                    nc.scalar.activation(MLOC1[:, mb, :], pm, AF.Copy)

                for j in range(SPC):
                    Mt = MSEL if j == 0 else MLOC1
                    for b in range(B):
                        i = bs_of(b, j)
                        st_i = pb.tile([P, NB, SEG], bf, tag="stp2", name=f"stp2_{i}")
                        nc.sync.dma_start(
                            out=st_i, in_=step_d[i].rearrange("kb p s -> p kb s"))
                        sq_i = sqT[i]
                        # denominator 1 x SEG
                        pd = psw.tile([1, SEG], f32, tag="dn")
                        for kb in range(NB):
                            nc.tensor.matmul(
                                pd, lhsT=ZCOL[:, kb, i:i + 1], rhs=sq_i[:, kb, :],
                                start=(kb == 0), stop=(kb == NB - 1),
                            )
                        rcm = pb.tile([1, SEG], bf, tag="rcm")
                        with nc.allow_low_precision(reason="bf16 memread recip"):
                            nc.vector.reciprocal(rcm, pd)
                        nc.sync.dma_start(out=rcm_d[i], in_=rcm)
                        rcmb = pb.tile([P, SEG], bf, tag="rcmb")
                        nc.gpsimd.dma_start(
                            out=rcmb,
                            in_=rcm_d[i:i + 1, :].partition_broadcast(P))
                        for mb in range(NB):
                            pm = psb.tile([P, SEG], f32, tag="mm")
                            for kb in range(NB):
                                nc.tensor.matmul(
                                    pm, lhsT=Mt[:, kb, mb * P:(mb + 1) * P],
                                    rhs=sq_i[:, kb, :],
                                    start=(kb == 0), stop=(kb == NB - 1),
                                )
                            mtmp = pb.tile([P, SEG], bf, tag="mt")
                            nc.vector.scalar_tensor_tensor(
                                out=mtmp, in0=pm, scalar=GC[:, mb:mb + 1],
                                in1=rcmb,
                                op0=OP.mult, op1=OP.mult,
                            )
                            nc.vector.tensor_add(st_i[:, mb, :], st_i[:, mb, :], mtmp)
                        for sb in range(NB):
                            po = psw.tile([P, D], f32, tag="wd")
                            for mb in range(NB):
                                nc.tensor.matmul(
                                    po, lhsT=st_i[:, mb, sb * P:(sb + 1) * P],
                                    rhs=WD[:, mb, :],
                                    start=(mb == 0), stop=(mb == NB - 1),
                                )
                            ob = pb.tile([P, D], f32, tag="ob")
                            nc.scalar.activation(ob, po, AF.Copy)
                            nc.sync.dma_start(
                                out=out_d[b, j, sb * P:(sb + 1) * P, :], in_=ob)


    nc.compile()
    return nc


def _prep_inputs(x, Wq, Wk, Wv, Wd, beta):
    """Host-side prep: transpose/cast/shard. Returns in_maps (list of 8 dicts)."""
    g = 1.0 / (1.0 + np.exp(-beta.astype(np.float64)))  # (H,)
    g = g.astype(np.float32)
    gcol = np.repeat(g, D).reshape(NB, P).T.copy()      # (P, NB): g[(kb*128+p)//64]
    omg = (1.0 - np.repeat(g, D)).reshape(NB, P).T.copy()

    def wprep(w):
        return np.ascontiguousarray(
            w.reshape(NB, P, w.shape[1]).astype(bf_np))

    wq_a, wk_a, wv_a = wprep(Wq), wprep(Wk), wprep(Wv)
    wd_a = wprep(Wd)
    cmask = np.triu(np.ones((P, P), np.float32)).astype(bf_np)
    ident = np.eye(P, dtype=np.float32).astype(bf_np)

    # x -> per-core transposed blocks: xt[b, j, kb, p, s] = x[b, (2c+j)*SEG+s, kb*P+p]
    xs = x.reshape(B, NSEG, SEG, DIN)
    in_maps = []
    for c in range(NC):
        xloc = xs[:, 2 * c:2 * c + 2]                        # (B, SPC, SEG, DIN)
        xt = xloc.transpose(0, 1, 3, 2)                      # (B, SPC, DIN, SEG)
        xt = np.ascontiguousarray(
            xt.reshape(B, SPC, NB, P, SEG).astype(bf_np))
        # AG1 global row for (t, b): rank t//2 contributes row (t%2)*B + b
        zmask = np.zeros((64, NC), np.float32)
        for jj in range(NC):
            tgt = 2 * c + (jj // B)
            bb = jj % B
            for t in range(NSEG):
                if t < tgt:
                    zmask[(t // 2) * BS + (t % 2) * B + bb, jj] = 1.0
        oh = np.zeros((P, NC), np.float32)
        if c >= 1:
            oh[:, c - 1] = 1.0
        in_maps.append({
            "xt": xt, "wq": wq_a, "wk": wk_a, "wv": wv_a, "wd": wd_a,
            "gcol": gcol, "omg": omg, "zmask": zmask, "oh": oh,
            "cmask": cmask, "ident": ident,
        })
    return in_maps


def kernel(x, Wq, Wk, Wv, Wd, beta, _trace=False):
    x = np.asarray(x, np.float32)
    in_maps = _prep_inputs(
        x, np.asarray(Wq, np.float32), np.asarray(Wk, np.float32),
        np.asarray(Wv, np.float32), np.asarray(Wd, np.float32),
        np.asarray(beta, np.float32))
    if "nc" not in _CACHE:
        _CACHE["nc"] = _build()
    nc = _CACHE["nc"]
    res = bass_utils.run_bass_kernel_spmd(
        nc, in_maps, core_ids=list(range(NC)), trace=_trace)
    _CACHE["last_results"] = res
    out = np.empty((B, L, D), np.float32)
    for c in range(NC):
        oc = res.results[c]["out"]                  # (B, SPC, SEG, D)
        out[:, 2 * c * SEG:(2 * c + 2) * SEG, :] = oc.reshape(B, SPC * SEG, D)
    return out



# revision 3
# speedup vs baseline: 1.6709x; 1.6709x over previous
"""Trainium2 Bass kernel for nn_MMHA_78039555768536.

Gated mix of per-segment causal softmax attention and a linear-attention
memory (delta rule, memory summed over batch per segment).

Strategy (8 cores): core c owns segments {2c, 2c+1} for all batches.
The memory recurrence M_{t+1} = A_t M_t + B_t (A_t = I - sk^T diag(1/d) sk,
symmetric) is never materialized: the only reads of M are
  out_mem = (sq @ M @ diag(g) @ Wd) / (sq @ z)
so we chain R = M @ GWd (HD x 64) instead, fed by Btil_t = sk^T vtil with
vtil = x @ (Wv diag(g) Wd) folded on the host.  Attention output is
token-major with Wd + (1-g) folded into Wv' on the host, so softmax
denominators land as psum columns (cheap partition-parallel reciprocals)
and no output projection matmul is needed at all.

Collectives: AG1 gathers per-segment colsums of sk (z prefix, 16 KB),
AG2 gathers the per-core composed pair (Abar^T, Btil-bar) (576 KB/rank).
Both overlap compute: AG1 behind vtil/va' projections, AG2 behind the
whole attention phase.
"""

import os
import sys

sys.path.insert(0, "/opt/trn_rl_repo")

from contextlib import ExitStack

import numpy as np
import ml_dtypes

import concourse.bass as bass
import concourse.bacc as bacc
import concourse.tile as tile
from concourse import mybir
from concourse import bass_utils

B, L, DIN = 4, 8192, 512
H, D, SEG = 8, 64, 512
HD = H * D
NSEG = L // SEG          # 16
NC = 8                   # cores
SPC = NSEG // NC         # segments per core = 2
P = 128
NB = HD // P             # 4 blocks of 128
BS = B * SPC             # batch-segment units per core = 8
AB_SZ = HD * HD + HD * D # AG2 payload elems per rank

bf = mybir.dt.bfloat16
f32 = mybir.dt.float32
AF = mybir.ActivationFunctionType
OP = mybir.AluOpType
bf_np = ml_dtypes.bfloat16

_CACHE = {}


def _build():
    nc = bacc.Bacc(
        "TRN2",
        target_bir_lowering=False,
        debug=False,
        enable_asserts=False,
        num_devices=NC,
    )

    # ---------------- DRAM I/O ----------------
    xt_d = nc.dram_tensor("xt", [B, SPC, NB, P, SEG], bf, kind="ExternalInput").ap()
    wq_d = nc.dram_tensor("wq", [NB, P, HD], bf, kind="ExternalInput").ap()
    wk_d = nc.dram_tensor("wk", [NB, P, HD], bf, kind="ExternalInput").ap()
    wvp_d = nc.dram_tensor("wvp", [NB, P, HD], bf, kind="ExternalInput").ap()
    wvt_d = nc.dram_tensor("wvt", [NB, P, D], bf, kind="ExternalInput").ap()
    zmask_d = nc.dram_tensor("zmask", [64, NC], f32, kind="ExternalInput").ap()
    oh_d = nc.dram_tensor("oh", [P, NC], f32, kind="ExternalInput").ap()
    mask_d = nc.dram_tensor("cmask", [P, P], bf, kind="ExternalInput").ap()
    ident_d = nc.dram_tensor("ident", [P, P], bf, kind="ExternalInput").ap()
    out_d = nc.dram_tensor("out", [B, SPC, SEG, D], f32, kind="ExternalOutput").ap()

    with tile.TileContext(nc) as tc, ExitStack() as ctx:
        const = ctx.enter_context(tc.tile_pool(name="const", bufs=1))
        dram = ctx.enter_context(tc.tile_pool(name="dram", bufs=1, space="DRAM"))
        keepx = ctx.enter_context(tc.tile_pool(name="keepx", bufs=BS))
        keepq = ctx.enter_context(tc.tile_pool(name="keepq", bufs=BS))
        kacc = ctx.enter_context(tc.tile_pool(name="kacc", bufs=BS))
        kvap = ctx.enter_context(tc.tile_pool(name="kvap", bufs=BS))
        phb = ctx.enter_context(tc.tile_pool(name="phb", bufs=1))

        WQ = const.tile([P, NB, HD], bf)
        WK = const.tile([P, NB, HD], bf)
        WVP = const.tile([P, NB, HD], bf)
        WVT = const.tile([P, NB, D], bf)
        ZM = const.tile([64, NC], f32)
        OH = const.tile([P, NC], f32)
        CM = const.tile([P, P], bf)
        ID = const.tile([P, P], bf)
        ONE = const.tile([P, 1], bf)

        nc.sync.dma_start(out=WQ, in_=wq_d.rearrange("kb p n -> p kb n"))
        nc.sync.dma_start(out=WK, in_=wk_d.rearrange("kb p n -> p kb n"))
        nc.sync.dma_start(out=WVP, in_=wvp_d.rearrange("kb p n -> p kb n"))
        nc.sync.dma_start(out=WVT, in_=wvt_d.rearrange("kb p n -> p kb n"))
        nc.sync.dma_start(out=ZM, in_=zmask_d)
        nc.sync.dma_start(out=OH, in_=oh_d)
        nc.sync.dma_start(out=CM, in_=mask_d)
        nc.sync.dma_start(out=ID, in_=ident_d)
        nc.vector.memset(ONE, 1.0)

        # collective bounce buffers
        cs_in = dram.tile([BS, HD], f32)
        cs_out = dram.tile([NC * BS, HD], f32, addr_space="Shared")
        zrow_d = dram.tile([BS, HD], bf)
        ab_in = dram.tile([AB_SZ], bf)
        ab_out = dram.tile([NC * AB_SZ], bf, addr_space="Shared")

        # long-lived SBUF
        XT = [keepx.tile([P, NB, SEG], bf, tag="xt", name=f"xt{i}") for i in range(BS)]
        SQ = [keepq.tile([P, NB, SEG], bf, tag="sq", name=f"sq{i}") for i in range(BS)]
        ACC = [kacc.tile([P, NB, D], f32, tag="acc", name=f"acc{i}") for i in range(BS)]
        VAP = [kvap.tile([P, NB, H, D + 1], bf, tag="vap", name=f"vap{i}")
               for i in range(BS)]

        ZROW = phb.tile([BS, HD], f32)
        ZCOL = phb.tile([P, NB, BS], bf)
        AT0 = phb.tile([P, NB, HD], bf)    # segment-2c A-part (= -K), retained
        BTT0 = phb.tile([P, NB, D], bf)    # segment-2c Btil, retained
        RSEL = phb.tile([P, NB, D], bf)    # selected R at segment 2c
        RLOC1 = phb.tile([P, NB, D], bf)   # R at segment 2c+1

        def bs_of(b, j):
            return j * B + b

        rgroups = [list(range(NC))]

        # ============ A phase ============
        with tc.tile_pool(name="psk", bufs=BS) as psk, \
             tc.tile_pool(name="pvt", bufs=BS) as pvtp, \
             tc.tile_pool(name="pskd", bufs=B) as pskd, \
             tc.tile_pool(name="pa", bufs=3) as pa, \
             tc.tile_pool(name="pab", bufs=1) as pab:
            skT = [psk.tile([P, NB, HD], bf, tag="sk", name=f"sk{i}") for i in range(BS)]
            vtT = [pvtp.tile([P, NB, D], bf, tag="vt", name=f"vt{i}") for i in range(BS)]

            # ---- A1: k-proj, sk=elu1(k), colsums ----
            with tc.tile_pool(name="ps1", bufs=3, space="PSUM") as ps1, \
                 tc.tile_pool(name="psc1", bufs=1, space="PSUM") as psc1:
                for j in range(SPC):
                    for b in range(B):
                        i = bs_of(b, j)
                        nc.sync.dma_start(
                            out=XT[i], in_=xt_d[b, j].rearrange("kb p s -> p kb s"))
                        for sb in range(NB):
                            pk = ps1.tile([P, SEG], f32, tag="pk")
                            for kb in range(NB):
                                nc.tensor.matmul(
                                    pk,
                                    lhsT=XT[i][:, kb, sb * P:(sb + 1) * P],
                                    rhs=WK[:, kb, :],
                                    start=(kb == 0),
                                    stop=(kb == NB - 1),
                                )
                            # elu1(k) = max(k + 1, exp(min(k, 0)))
                            em = pa.tile([P, SEG], bf, tag="em")
                            nc.vector.tensor_scalar_min(em, pk, 0.0)
                            ee = pa.tile([P, SEG], bf, tag="ee")
                            nc.scalar.activation(ee, em, AF.Exp)
                            nc.vector.scalar_tensor_tensor(
                                out=skT[i][:, sb, :], in0=pk, scalar=1.0, in1=ee,
                                op0=OP.add, op1=OP.max,
                            )
                        pc = psc1.tile([1, HD], f32, tag="pc")
                        for sb in range(NB):
                            nc.tensor.matmul(
                                pc, lhsT=ONE, rhs=skT[i][:, sb, :],
                                start=(sb == 0), stop=(sb == NB - 1),
                            )
                        cs_sb = pa.tile([1, HD], f32, tag="cs")
                        nc.scalar.activation(cs_sb, pc, AF.Copy)
                        nc.sync.dma_start(out=cs_in[i:i + 1, :], in_=cs_sb)

            # ---- AG1 (async; overlapped by vtil/va' projections) ----
            nc.gpsimd.collective_compute(
                "AllGather", OP.bypass,
                replica_groups=rgroups,
                ins=[cs_in.opt()], outs=[cs_out.opt()],
            )

            # ---- vtil + va' projections (no AG1 dependency) ----
            with tc.tile_pool(name="psv", bufs=2, space="PSUM") as psv, \
                 tc.tile_pool(name="psvt", bufs=2, space="PSUM") as psvt:
                for j in range(SPC):
                    for b in range(B):
                        i = bs_of(b, j)
                        nc.vector.memset(VAP[i][:, :, :, D:D + 1], 1.0)
                        for sb in range(NB):
                            pvp = psv.tile([P, SEG], f32, tag="pvp")
                            pvt = psvt.tile([P, D], f32, tag="pvt")
                            for kb in range(NB):
                                nc.tensor.matmul(
                                    pvp,
                                    lhsT=XT[i][:, kb, sb * P:(sb + 1) * P],
                                    rhs=WVP[:, kb, :],
                                    start=(kb == 0), stop=(kb == NB - 1),
                                )
                            for kb in range(NB):
                                nc.tensor.matmul(
                                    pvt,
                                    lhsT=XT[i][:, kb, sb * P:(sb + 1) * P],
                                    rhs=WVT[:, kb, :],
                                    start=(kb == 0), stop=(kb == NB - 1),
                                )
                            nc.vector.tensor_copy(
                                VAP[i][:, sb, :, 0:D],
                                pvp.rearrange("p (h d) -> p h d", h=H),
                            )
                            nc.scalar.activation(vtT[i][:, sb, :], pvt, AF.Copy)

            # ---- z prefix (needs AG1) ----
            with tc.tile_pool(name="pz", bufs=1) as pz, \
                 tc.tile_pool(name="psz", bufs=2, space="PSUM") as psz:
                Z = pz.tile([NC * BS, HD], f32, tag="z")
                nc.sync.dma_start(out=Z, in_=cs_out)
                zp = psz.tile([BS, HD], f32, tag="zp")
                nc.tensor.matmul(zp, lhsT=ZM, rhs=Z, start=True, stop=True)
                nc.scalar.activation(ZROW, zp, AF.Copy, bias=1.0 / D)
                ZROW16 = pz.tile([BS, HD], bf, tag="zr16")
                nc.vector.tensor_copy(ZROW16, ZROW)
                nc.sync.dma_start(out=zrow_d, in_=ZROW16)
                for kb in range(NB):
                    zc = psz.tile([P, BS], f32, tag="zc")
                    nc.tensor.matmul(zc, lhsT=Z[:, kb * P:(kb + 1) * P], rhs=ZM,
                                     start=True, stop=True)
                    nc.scalar.activation(ZCOL[:, kb, :], zc, AF.Copy, bias=1.0 / D)

            # ---- d, skd, A-part + Btil per segment; compose; AG2 ----
            with tc.tile_pool(name="psA", bufs=2, space="PSUM") as psA, \
                 tc.tile_pool(name="psBt", bufs=2, space="PSUM") as psBt:
                at1 = btt1 = None
                for j in range(SPC):
                    skd = [None] * B
                    for b in range(B):
                        i = bs_of(b, j)
                        zbp = pa.tile([P, HD], bf, tag="zbp")
                        nc.gpsimd.dma_start(
                            out=zbp,
                            in_=zrow_d[i:i + 1, :].partition_broadcast(P))
                        jnk = pa.tile([P, HD], bf, tag="jnk")
                        dcol = pa.tile([P, NB], f32, tag="dcol")
                        for sb in range(NB):
                            nc.vector.scalar_tensor_tensor(
                                out=jnk, in0=skT[i][:, sb, :], scalar=1.0, in1=zbp,
                                op0=OP.mult, op1=OP.mult,
                                accum_out=dcol[:, sb:sb + 1],
                            )
                        rcd = pa.tile([P, NB], f32, tag="rcd")
                        nc.vector.reciprocal(rcd, dcol)
                        sd = pskd.tile([P, NB, HD], bf, tag="skd")
                        skd[b] = sd
                        for sb in range(NB):
                            nc.vector.tensor_scalar_mul(
                                sd[:, sb, :], skT[i][:, sb, :], rcd[:, sb:sb + 1])

                    at_t = AT0 if j == 0 else pab.tile([P, NB, HD], bf, tag="at1")
                    btt_t = BTT0 if j == 0 else pab.tile([P, NB, D], bf, tag="btt1")
                    for mb in range(NB):
                        pA = psA.tile([P, HD], f32, tag="pA")
                        pBt = psBt.tile([P, D], f32, tag="pBt")
                        n = 0
                        for b in range(B):
                            for sb in range(NB):
                                i = bs_of(b, j)
                                lhsT = skT[i][:, sb, mb * P:(mb + 1) * P]
                                nc.tensor.matmul(
                                    pA, lhsT=lhsT, rhs=skd[b][:, sb, :],
                                    start=(n == 0), stop=(n == B * NB - 1))
                                nc.tensor.matmul(
                                    pBt, lhsT=lhsT, rhs=vtT[i][:, sb, :],
                                    start=(n == 0), stop=(n == B * NB - 1))
                                n += 1
                        # A-part = -K
                        nc.scalar.activation(at_t[:, mb, :], pA, AF.Copy, scale=-1.0)
                        nc.vector.tensor_copy(btt_t[:, mb, :], pBt)
                    if j > 0:
                        at1, btt1 = at_t, btt_t

                # pair composition: abA = (Abar-I)^T = a0 a1 + a0 + a1
                #                   abBt = a1 bt0 + bt0 + bt1
                abA = pab.tile([P, NB, HD], bf, tag="abA")
                abBt = pab.tile([P, NB, D], bf, tag="abBt")
                for mb in range(NB):
                    pX = psA.tile([P, HD], f32, tag="pA")
                    for kb in range(NB):
                        nc.tensor.matmul(
                            pX, lhsT=AT0[:, kb, mb * P:(mb + 1) * P],
                            rhs=at1[:, kb, :], start=(kb == 0), stop=False)
                    nc.tensor.matmul(pX, lhsT=ID, rhs=AT0[:, mb, :],
                                     start=False, stop=False)
                    nc.tensor.matmul(pX, lhsT=ID, rhs=at1[:, mb, :],
                                     start=False, stop=True)
                    nc.scalar.activation(abA[:, mb, :], pX, AF.Copy)
                for mb in range(NB):
                    pY = psBt.tile([P, D], f32, tag="pBt")
                    for kb in range(NB):
                        nc.tensor.matmul(
                            pY, lhsT=at1[:, kb, mb * P:(mb + 1) * P],
                            rhs=BTT0[:, kb, :], start=(kb == 0), stop=False)
                    nc.tensor.matmul(pY, lhsT=ID, rhs=BTT0[:, mb, :],
                                     start=False, stop=False)
                    nc.tensor.matmul(pY, lhsT=ID, rhs=btt1[:, mb, :],
                                     start=False, stop=True)
                    nc.scalar.activation(abBt[:, mb, :], pY, AF.Copy)
                nc.sync.dma_start(
                    out=ab_in[0:HD * HD].rearrange("(kb p n) -> p kb n", p=P, n=HD),
                    in_=abA)
                nc.sync.dma_start(
                    out=ab_in[HD * HD:AB_SZ].rearrange("(kb p n) -> p kb n", p=P, n=D),
                    in_=abBt)

        # ---- AG2 (async; overlapped by the whole attention phase) ----
        nc.gpsimd.collective_compute(
            "AllGather", OP.bypass,
            replica_groups=rgroups,
            ins=[ab_in.opt()], outs=[ab_out.opt()],
        )

        # ============ attention phase ============
        with tc.tile_pool(name="patt", bufs=3) as patt, \
             tc.tile_pool(name="pqk", bufs=2) as pqk, \
             tc.tile_pool(name="pwt", bufs=16) as pwt, \
             tc.tile_pool(name="psp", bufs=2, space="PSUM") as psp, \
             tc.tile_pool(name="pssc", bufs=4, space="PSUM") as pssc, \
             tc.tile_pool(name="psat", bufs=2, space="PSUM") as psat:
            for j in range(SPC):
                for b in range(B):
                    i = bs_of(b, j)
                    qh = pqk.tile([P, NB, SEG], bf, tag="qh")
                    for mb in range(NB):
                        pq = psp.tile([P, SEG], f32, tag="pp")
                        for kb in range(NB):
                            nc.tensor.matmul(
                                pq, lhsT=WQ[:, kb, mb * P:(mb + 1) * P],
                                rhs=XT[i][:, kb, :],
                                start=(kb == 0), stop=(kb == NB - 1))
                        nc.scalar.activation(qh[:, mb, :], pq, AF.Copy)
                        em = patt.tile([P, SEG], bf, tag="em")
                        nc.vector.tensor_scalar_min(em, pq, 0.0)
                        ee = patt.tile([P, SEG], bf, tag="ee")
                        nc.scalar.activation(ee, em, AF.Exp)
                        nc.vector.scalar_tensor_tensor(
                            out=SQ[i][:, mb, :], in0=pq, scalar=1.0, in1=ee,
                            op0=OP.add, op1=OP.max)
                    kh = pqk.tile([P, NB, SEG], bf, tag="kh")
                    for mb in range(NB):
                        pkt = psp.tile([P, SEG], f32, tag="pp")
                        for kb in range(NB):
                            nc.tensor.matmul(
                                pkt, lhsT=WK[:, kb, mb * P:(mb + 1) * P],
                                rhs=XT[i][:, kb, :],
                                start=(kb == 0), stop=(kb == NB - 1))
                        nc.scalar.activation(kh[:, mb, :], pkt, AF.Copy)

                    for hb in range(NB):
                        wts = [[None] * NB, [None] * NB]
                        for kb in range(NB):
                            q0 = kb * P
                            qf = SEG - q0
                            for hh in range(2):
                                ho = hh * 64
                                ps_ = pssc.tile([P, SEG], f32, tag="sc")
                                nc.tensor.matmul(
                                    ps_[:, 0:qf],
                                    lhsT=kh[ho:ho + 64, hb, q0:q0 + P],
                                    rhs=qh[ho:ho + 64, hb, q0:SEG],
                                    start=True, stop=True)
                                wt = pwt.tile([P, SEG], bf, tag="wt")
                                nc.scalar.activation(wt[:, 0:qf], ps_[:, 0:qf],
                                                     AF.Exp, scale=0.125)
                                nc.vector.tensor_mul(wt[:, 0:P], wt[:, 0:P], CM)
                                wts[hh][kb] = wt
                        for hh in range(2):
                            h = 2 * hb + hh
                            for sb in range(NB):
                                pat = psat.tile([P, D + 1], f32, tag="pat")
                                for kb in range(sb + 1):
                                    nc.tensor.matmul(
                                        pat,
                                        lhsT=wts[hh][kb][:, (sb - kb) * P:(sb - kb + 1) * P],
                                        rhs=VAP[i][:, kb, h, :],
                                        start=(kb == 0), stop=(kb == sb))
                                rc = patt.tile([P, 1], f32, tag="rc")
                                nc.vector.reciprocal(rc, pat[:, D:D + 1])
                                if h == 0:
                                    nc.vector.tensor_scalar_mul(
                                        ACC[i][:, sb, :], pat[:, 0:D], rc)
                                else:
                                    nc.vector.scalar_tensor_tensor(
                                        out=ACC[i][:, sb, :], in0=pat[:, 0:D],
                                        scalar=rc, in1=ACC[i][:, sb, :],
                                        op0=OP.mult, op1=OP.add)

        # ============ chain on R (needs AG2) ============
        nc.vector.memset(RSEL, 0.0)
        with tc.tile_pool(name="pch", bufs=2) as pch, \
             tc.tile_pool(name="psch", bufs=NB, space="PSUM") as psch, \
             tc.tile_pool(name="psl", bufs=1, space="PSUM") as psl:
            pR = [psch.tile([P, D], f32, tag="ch", name=f"chain{m}") for m in range(NB)]
            rprev = None
            for step in range(NC - 1):
                base = step * AB_SZ
                cA = pch.tile([P, NB, HD], bf, tag="cA")
                cBt = pch.tile([P, NB, D], bf, tag="cBt")
                nc.sync.dma_start(
                    out=cA,
                    in_=ab_out[base:base + HD * HD].rearrange(
                        "(kb p n) -> p kb n", p=P, n=HD))
                nc.sync.dma_start(
                    out=cBt,
                    in_=ab_out[base + HD * HD:base + AB_SZ].rearrange(
                        "(kb p n) -> p kb n", p=P, n=D))
                rcur = pch.tile([P, NB, D], bf, tag="rcur")
                for mb in range(NB):
                    if step == 0:
                        nc.tensor.matmul(pR[mb], lhsT=ID, rhs=cBt[:, mb, :],
                                         start=True, stop=True)
                    else:
                        for kb in range(NB):
                            nc.tensor.matmul(
                                pR[mb], lhsT=cA[:, kb, mb * P:(mb + 1) * P],
                                rhs=rprev[:, kb, :], start=False, stop=False)
                        nc.tensor.matmul(pR[mb], lhsT=ID, rhs=cBt[:, mb, :],
                                         start=False, stop=True)
                    nc.scalar.activation(rcur[:, mb, :], pR[mb], AF.Copy)
                    nc.vector.scalar_tensor_tensor(
                        out=RSEL[:, mb, :], in0=rcur[:, mb, :],
                        scalar=OH[:, step:step + 1], in1=RSEL[:, mb, :],
                        op0=OP.mult, op1=OP.add)
                rprev = rcur

            # R at segment 2c+1: RLOC1 = a0 RSEL + RSEL + BTT0
            for mb in range(NB):
                pRl = psl.tile([P, D], f32, tag="pRl")
                for kb in range(NB):
                    nc.tensor.matmul(
                        pRl, lhsT=AT0[:, kb, mb * P:(mb + 1) * P],
                        rhs=RSEL[:, kb, :], start=(kb == 0), stop=False)
                nc.tensor.matmul(pRl, lhsT=ID, rhs=RSEL[:, mb, :],
                                 start=False, stop=False)
                nc.tensor.matmul(pRl, lhsT=ID, rhs=BTT0[:, mb, :],
                                 start=False, stop=True)
                nc.scalar.activation(RLOC1[:, mb, :], pRl, AF.Copy)

        # ============ phase B: mem path + combine + store ============
        with tc.tile_pool(name="pb", bufs=3) as pb, \
             tc.tile_pool(name="pso", bufs=2, space="PSUM") as pso, \
             tc.tile_pool(name="psd", bufs=2, space="PSUM") as psd:
            for j in range(SPC):
                Rt = RSEL if j == 0 else RLOC1
                for b in range(B):
                    i = bs_of(b, j)
                    for sb in range(NB):
                        po = pso.tile([P, D], f32, tag="po")
                        pd = psd.tile([P, 1], f32, tag="pd")
                        for kb in range(NB):
                            lhsT = SQ[i][:, kb, sb * P:(sb + 1) * P]
                            nc.tensor.matmul(
                                po, lhsT=lhsT, rhs=Rt[:, kb, :],
                                start=(kb == 0), stop=(kb == NB - 1))
                            nc.tensor.matmul(
                                pd, lhsT=lhsT, rhs=ZCOL[:, kb, i:i + 1],
                                start=(kb == 0), stop=(kb == NB - 1))
                        rcm = pb.tile([P, 1], f32, tag="rcm")
                        nc.vector.reciprocal(rcm, pd)
                        ob = pb.tile([P, D], f32, tag="ob")
                        nc.vector.scalar_tensor_tensor(
                            out=ob, in0=po, scalar=rcm, in1=ACC[i][:, sb, :],
                            op0=OP.mult, op1=OP.add)
                        nc.sync.dma_start(
                            out=out_d[b, j, sb * P:(sb + 1) * P, :], in_=ob)

    nc.compile()
    return nc


def _prep_inputs(x, Wq, Wk, Wv, Wd, beta):
    """Host-side prep: fold gates/Wd into Wv, transpose/cast/shard."""
    g = 1.0 / (1.0 + np.exp(-beta.astype(np.float64)))  # (H,)
    g = g.astype(np.float32)
    grep = np.repeat(g, D)

    # Wv' : attention value proj with (1-g_h) Wd[h-block] folded per head
    Wvp = np.zeros((DIN, HD), np.float32)
    for h in range(H):
        Wvp[:, h * D:(h + 1) * D] = (
            Wv[:, h * D:(h + 1) * D] @ ((1.0 - g[h]) * Wd[h * D:(h + 1) * D]))
    # Wvt : vtil proj  (v @ diag(g) Wd  folded)
    Wvt = (Wv * grep[None, :]) @ Wd  # (DIN, D)

    def wprep(w):
        return np.ascontiguousarray(w.reshape(NB, P, w.shape[1]).astype(bf_np))

    wq_a, wk_a = wprep(Wq), wprep(Wk)
    wvp_a, wvt_a = wprep(Wvp), wprep(Wvt)
    cmask = np.triu(np.ones((P, P), np.float32)).astype(bf_np)
    ident = np.eye(P, dtype=np.float32).astype(bf_np)

    xs = x.reshape(B, NSEG, SEG, DIN)
    in_maps = []
    for c in range(NC):
        xloc = xs[:, 2 * c:2 * c + 2]                        # (B, SPC, SEG, DIN)
        xt = xloc.transpose(0, 1, 3, 2)                      # (B, SPC, DIN, SEG)
        xt = np.ascontiguousarray(
            xt.reshape(B, SPC, NB, P, SEG).astype(bf_np))
        # AG1 global row for (t, b): rank t//2 contributes row (t%2)*B + b
        zmask = np.zeros((64, NC), np.float32)
        for jj in range(NC):
            tgt = 2 * c + (jj // B)
            bb = jj % B
            for t in range(NSEG):
                if t < tgt:
                    zmask[(t // 2) * BS + (t % 2) * B + bb, jj] = 1.0
        oh = np.zeros((P, NC), np.float32)
        if c >= 1:
            oh[:, c - 1] = 1.0
        in_maps.append({
            "xt": xt, "wq": wq_a, "wk": wk_a, "wvp": wvp_a, "wvt": wvt_a,
            "zmask": zmask, "oh": oh, "cmask": cmask, "ident": ident,
        })
    return in_maps


def kernel(x, Wq, Wk, Wv, Wd, beta, _trace=False):
    x = np.asarray(x, np.float32)
    in_maps = _prep_inputs(
        x, np.asarray(Wq, np.float32), np.asarray(Wk, np.float32),
        np.asarray(Wv, np.float32), np.asarray(Wd, np.float32),
        np.asarray(beta, np.float32))
    if "nc" not in _CACHE:
        _CACHE["nc"] = _build()
    nc = _CACHE["nc"]
    res = bass_utils.run_bass_kernel_spmd(
        nc, in_maps, core_ids=list(range(NC)), trace=_trace)
    _CACHE["last_results"] = res
    out = np.empty((B, L, D), np.float32)
    for c in range(NC):
        oc = res.results[c]["out"]                  # (B, SPC, SEG, D)
        out[:, 2 * c * SEG:(2 * c + 2) * SEG, :] = oc.reshape(B, SPC * SEG, D)
    return out


# revision 9
# speedup vs baseline: 1.8808x; 1.1256x over previous
"""Trainium2 Bass kernel for nn_MMHA_78039555768536.

Gated mix of per-segment causal softmax attention and a linear-attention
memory (delta rule, memory summed over batch per segment).

Strategy (8 cores): core c owns segments {2c, 2c+1} for all batches.
The memory recurrence M_{t+1} = A_t M_t + B_t (A_t = I - sk^T diag(1/d) sk,
symmetric) is never materialized: the only reads of M are
  out_mem = (sq @ M @ diag(g) @ Wd) / (sq @ z)
so we chain R = M @ GWd (HD x 64) instead, fed by Btil_t = sk^T vtil with
vtil = x @ (Wv diag(g) Wd) folded on the host.  Attention output is
token-major with Wd + (1-g) folded into Wv' on the host, so softmax
denominators land as psum columns (cheap partition-parallel reciprocals)
and no output projection matmul is needed at all.

Collectives: AG1 gathers per-segment colsums of sk (z prefix, 16 KB),
AG2 gathers the per-core composed pair (Abar^T, Btil-bar) (576 KB/rank).
Both overlap compute; the R-chain is interleaved into the attention
phase so only the small phase-B matmuls trail it.
"""

import os
import sys

sys.path.insert(0, "/opt/trn_rl_repo")

from contextlib import ExitStack

import numpy as np
import ml_dtypes

import concourse.bass as bass
import concourse.bacc as bacc
import concourse.tile as tile
from concourse import mybir
from concourse import bass_utils

B, L, DIN = 4, 8192, 512
H, D, SEG = 8, 64, 512
HD = H * D
NSEG = L // SEG          # 16
NC = 8                   # cores
SPC = NSEG // NC         # segments per core = 2
P = 128
NB = HD // P             # 4 blocks of 128
BS = B * SPC             # batch-segment units per core = 8
AB_SZ = HD * HD + HD * D # AG2 payload elems per rank

bf = mybir.dt.bfloat16
f32 = mybir.dt.float32
AF = mybir.ActivationFunctionType
OP = mybir.AluOpType
bf_np = ml_dtypes.bfloat16

_CACHE = {}


def _build():
    nc = bacc.Bacc(
        "TRN2",
        target_bir_lowering=False,
        debug=False,
        enable_asserts=False,
        num_devices=NC,
    )

    # ---------------- DRAM I/O ----------------
    xt_d = nc.dram_tensor("xt", [B, SPC, NB, P, SEG], bf, kind="ExternalInput").ap()
    wq_d = nc.dram_tensor("wq", [NB, P, HD], bf, kind="ExternalInput").ap()
    wk_d = nc.dram_tensor("wk", [NB, P, HD], bf, kind="ExternalInput").ap()
    wvp_d = nc.dram_tensor("wvp", [NB, P, HD], bf, kind="ExternalInput").ap()
    wvt_d = nc.dram_tensor("wvt", [NB, P, D], bf, kind="ExternalInput").ap()
    zmask_d = nc.dram_tensor("zmask", [64, NC], f32, kind="ExternalInput").ap()
    oh_d = nc.dram_tensor("oh", [P, NC], f32, kind="ExternalInput").ap()
    mask_d = nc.dram_tensor("cmask", [P, P], bf, kind="ExternalInput").ap()
    ident_d = nc.dram_tensor("ident", [P, P], bf, kind="ExternalInput").ap()
    out_d = nc.dram_tensor("out", [B, SPC, SEG, D], f32, kind="ExternalOutput").ap()

    with tile.TileContext(nc) as tc, ExitStack() as ctx:
        const = ctx.enter_context(tc.tile_pool(name="const", bufs=1))
        dram = ctx.enter_context(tc.tile_pool(name="dram", bufs=1, space="DRAM"))
        keepx = ctx.enter_context(tc.tile_pool(name="keepx", bufs=BS))
        keepq = ctx.enter_context(tc.tile_pool(name="keepq", bufs=BS))
        kacc = ctx.enter_context(tc.tile_pool(name="kacc", bufs=BS))
        kvap = ctx.enter_context(tc.tile_pool(name="kvap", bufs=BS))
        phb = ctx.enter_context(tc.tile_pool(name="phb", bufs=1))

        WQ = const.tile([P, NB, HD], bf)
        WK = const.tile([P, NB, HD], bf)
        WVP = const.tile([P, NB, HD], bf)
        WVT = const.tile([P, NB, D], bf)
        ZM = const.tile([64, NC], f32)
        OH = const.tile([P, NC], f32)
        CM = const.tile([P, P], bf)
        ID = const.tile([P, P], bf)
        ONE = const.tile([P, 1], bf)

        nc.sync.dma_start(out=WK, in_=wk_d.rearrange("kb p n -> p kb n"))
        nc.sync.dma_start(out=WVP, in_=wvp_d.rearrange("kb p n -> p kb n"))
        nc.sync.dma_start(out=WVT, in_=wvt_d.rearrange("kb p n -> p kb n"))
        nc.sync.dma_start(out=WQ, in_=wq_d.rearrange("kb p n -> p kb n"))
        nc.sync.dma_start(out=ZM, in_=zmask_d)
        nc.sync.dma_start(out=OH, in_=oh_d)
        nc.sync.dma_start(out=CM, in_=mask_d)
        nc.sync.dma_start(out=ID, in_=ident_d)
        nc.vector.memset(ONE, 1.0)

        # collective bounce buffers
        cs_in = dram.tile([BS, HD], f32)
        cs_out = dram.tile([NC * BS, HD], f32, addr_space="Shared")
        zrow_d = dram.tile([BS, HD], bf)
        ab_in = dram.tile([AB_SZ], bf)
        ab_out = dram.tile([NC * AB_SZ], bf, addr_space="Shared")

        # long-lived SBUF
        XT = [keepx.tile([P, NB, SEG], bf, tag="xt", name=f"xt{i}") for i in range(BS)]
        SQ = [keepq.tile([P, NB, SEG], bf, tag="sq", name=f"sq{i}") for i in range(BS)]
        ACC = [kacc.tile([P, NB, D], f32, tag="acc", name=f"acc{i}") for i in range(BS)]
        VAP = [kvap.tile([P, NB, H, D + 1], bf, tag="vap", name=f"vap{i}")
               for i in range(BS)]

        ZROW = phb.tile([BS, HD], f32)
        ZCOL = phb.tile([P, NB, BS], bf)
        AT0 = phb.tile([P, NB, HD], bf)    # segment-2c A-part (= -K), retained
        BTT0 = phb.tile([P, NB, D], bf)    # segment-2c Btil, retained
        RSEL = phb.tile([P, NB, D], bf)    # selected R at segment 2c
        RLOC1 = phb.tile([P, NB, D], bf)   # R at segment 2c+1

        def bs_of(b, j):
            return j * B + b

        order = [(j, b) for j in range(SPC) for b in range(B)]
        rgroups = [list(range(NC))]

        # ============ A phase ============
        with tc.tile_pool(name="psk", bufs=BS) as psk, \
             tc.tile_pool(name="pvt", bufs=BS) as pvtp, \
             tc.tile_pool(name="pskd", bufs=B) as pskd, \
             tc.tile_pool(name="pa", bufs=2) as pa, \
             tc.tile_pool(name="pab", bufs=1) as pab:
            skT = [psk.tile([P, NB, HD], bf, tag="sk", name=f"sk{i}") for i in range(BS)]
            vtT = [pvtp.tile([P, NB, D], bf, tag="vt", name=f"vt{i}") for i in range(BS)]

            # ---- A1: k-proj, sk=elu1(k), colsums (pipelined one behind) ----
            with tc.tile_pool(name="ps1", bufs=3, space="PSUM") as ps1, \
                 tc.tile_pool(name="psc1", bufs=2, space="PSUM") as psc1:
                def colsum(i):
                    pc = psc1.tile([1, HD], f32, tag="pc")
                    for sb in range(NB):
                        nc.tensor.matmul(
                            pc, lhsT=ONE, rhs=skT[i][:, sb, :],
                            start=(sb == 0), stop=(sb == NB - 1),
                        )
                    cs_sb = pa.tile([1, HD], f32, tag="cs")
                    nc.scalar.activation(cs_sb, pc, AF.Copy)
                    nc.sync.dma_start(out=cs_in[i:i + 1, :], in_=cs_sb)

                prev = None
                for j, b in order:
                    i = bs_of(b, j)
                    nc.scalar.dma_start(
                        out=XT[i], in_=xt_d[b, j].rearrange("kb p s -> p kb s"))
                    for sb in range(NB):
                        pk = ps1.tile([P, SEG], f32, tag="pk")
                        for kb in range(NB):
                            nc.tensor.matmul(
                                pk,
                                lhsT=XT[i][:, kb, sb * P:(sb + 1) * P],
                                rhs=WK[:, kb, :],
                                start=(kb == 0),
                                stop=(kb == NB - 1),
                            )
                        # elu1(k) = max(k + 1, exp(min(k, 0)))
                        em = pa.tile([P, SEG], bf, tag="em")
                        nc.vector.tensor_scalar_min(em, pk, 0.0)
                        ee = pa.tile([P, SEG], bf, tag="ee")
                        nc.scalar.activation(ee, em, AF.Exp)
                        nc.vector.scalar_tensor_tensor(
                            out=skT[i][:, sb, :], in0=pk, scalar=1.0, in1=ee,
                            op0=OP.add, op1=OP.max,
                        )
                    if prev is not None:
                        colsum(prev)
                    prev = i
                colsum(prev)

            # ---- AG1 (async; overlapped by vtil/va' projections) ----
            nc.gpsimd.collective_compute(
                "AllGather", OP.bypass,
                replica_groups=rgroups,
                ins=[cs_in.opt()], outs=[cs_out.opt()],
            )

            # ---- vtil + va' projections (no AG1 dependency) ----
            with tc.tile_pool(name="psv", bufs=3, space="PSUM") as psv, \
                 tc.tile_pool(name="psvt", bufs=2, space="PSUM") as psvt:
                for j, b in order:
                    i = bs_of(b, j)
                    nc.vector.memset(VAP[i][:, :, :, D:D + 1], 1.0)
                    for sb in range(NB):
                        pvp = psv.tile([P, SEG], f32, tag="pvp")
                        pvt = psvt.tile([P, D], f32, tag="pvt")
                        for kb in range(NB):
                            nc.tensor.matmul(
                                pvp,
                                lhsT=XT[i][:, kb, sb * P:(sb + 1) * P],
                                rhs=WVP[:, kb, :],
                                start=(kb == 0), stop=(kb == NB - 1),
                            )
                        for kb in range(NB):
                            nc.tensor.matmul(
                                pvt,
                                lhsT=XT[i][:, kb, sb * P:(sb + 1) * P],
                                rhs=WVT[:, kb, :],
                                start=(kb == 0), stop=(kb == NB - 1),
                            )
                        nc.vector.tensor_copy(
                            VAP[i][:, sb, :, 0:D],
                            pvp.rearrange("p (h d) -> p h d", h=H),
                        )
                        nc.scalar.activation(vtT[i][:, sb, :], pvt, AF.Copy)

            # ---- z prefix (needs AG1) ----
            with tc.tile_pool(name="pz", bufs=1) as pz, \
                 tc.tile_pool(name="psz", bufs=2, space="PSUM") as psz:
                Z = pz.tile([NC * BS, HD], f32, tag="z")
                nc.sync.dma_start(out=Z, in_=cs_out)
                zp = psz.tile([BS, HD], f32, tag="zp")
                nc.tensor.matmul(zp, lhsT=ZM, rhs=Z, start=True, stop=True)
                nc.scalar.activation(ZROW, zp, AF.Copy, bias=1.0 / D)
                ZROW16 = pz.tile([BS, HD], bf, tag="zr16")
                nc.vector.tensor_copy(ZROW16, ZROW)
                nc.sync.dma_start(out=zrow_d, in_=ZROW16)
                for kb in range(NB):
                    zc = psz.tile([P, BS], f32, tag="zc")
                    nc.tensor.matmul(zc, lhsT=Z[:, kb * P:(kb + 1) * P], rhs=ZM,
                                     start=True, stop=True)
                    nc.scalar.activation(ZCOL[:, kb, :], zc, AF.Copy, bias=1.0 / D)

            # ---- d, skd (gpsimd), A-part + Btil per segment; compose ----
            with tc.tile_pool(name="psA", bufs=2, space="PSUM") as psA, \
                 tc.tile_pool(name="psBt", bufs=2, space="PSUM") as psBt:
                at1 = btt1 = None
                for j in range(SPC):
                    skd = [None] * B
                    for b in range(B):
                        i = bs_of(b, j)
                        zbp = pa.tile([P, HD], bf, tag="zbp")
                        nc.gpsimd.dma_start(
                            out=zbp,
                            in_=zrow_d[i:i + 1, :].partition_broadcast(P))
                        jnk = pa.tile([P, HD], bf, tag="jnk")
                        dcol = pa.tile([P, NB], f32, tag="dcol")
                        for sb in range(NB):
                            nc.vector.scalar_tensor_tensor(
                                out=jnk, in0=skT[i][:, sb, :], scalar=1.0, in1=zbp,
                                op0=OP.mult, op1=OP.mult,
                                accum_out=dcol[:, sb:sb + 1],
                            )
                        rcd = pa.tile([P, NB], f32, tag="rcd")
                        nc.vector.reciprocal(rcd, dcol)
                        sd = pskd.tile([P, NB, HD], bf, tag="skd")
                        skd[b] = sd
                        for sb in range(NB):
                            nc.vector.tensor_scalar_mul(
                                sd[:, sb, :], skT[i][:, sb, :], rcd[:, sb:sb + 1])

                    at_t = AT0 if j == 0 else pab.tile([P, NB, HD], bf, tag="at1")
                    btt_t = BTT0 if j == 0 else pab.tile([P, NB, D], bf, tag="btt1")
                    for mb in range(NB):
                        pA = psA.tile([P, HD], f32, tag="pA")
                        pBt = psBt.tile([P, D], f32, tag="pBt")
                        n = 0
                        for b in range(B):
                            for sb in range(NB):
                                i = bs_of(b, j)
                                lhsT = skT[i][:, sb, mb * P:(mb + 1) * P]
                                nc.tensor.matmul(
                                    pA, lhsT=lhsT, rhs=skd[b][:, sb, :],
                                    start=(n == 0), stop=(n == B * NB - 1))
                                nc.tensor.matmul(
                                    pBt, lhsT=lhsT, rhs=vtT[i][:, sb, :],
                                    start=(n == 0), stop=(n == B * NB - 1))
                                n += 1
                        # A-part = -K
                        nc.scalar.activation(at_t[:, mb, :], pA, AF.Copy, scale=-1.0)
                        nc.vector.tensor_copy(btt_t[:, mb, :], pBt)
                    if j > 0:
                        at1, btt1 = at_t, btt_t

                # pair composition: abA = (Abar-I)^T = a0 a1 + a0 + a1
                #                   abBt = a1 bt0 + bt0 + bt1
                abA = pab.tile([P, NB, HD], bf, tag="abA")
                abBt = pab.tile([P, NB, D], bf, tag="abBt")
                for mb in range(NB):
                    pX = psA.tile([P, HD], f32, tag="pA")
                    for kb in range(NB):
                        nc.tensor.matmul(
                            pX, lhsT=AT0[:, kb, mb * P:(mb + 1) * P],
                            rhs=at1[:, kb, :], start=(kb == 0), stop=False)
                    nc.tensor.matmul(pX, lhsT=ID, rhs=AT0[:, mb, :],
                                     start=False, stop=False)
                    nc.tensor.matmul(pX, lhsT=ID, rhs=at1[:, mb, :],
                                     start=False, stop=True)
                    nc.scalar.activation(abA[:, mb, :], pX, AF.Copy)
                for mb in range(NB):
                    pY = psBt.tile([P, D], f32, tag="pBt")
                    for kb in range(NB):
                        nc.tensor.matmul(
                            pY, lhsT=at1[:, kb, mb * P:(mb + 1) * P],
                            rhs=BTT0[:, kb, :], start=(kb == 0), stop=False)
                    nc.tensor.matmul(pY, lhsT=ID, rhs=BTT0[:, mb, :],
                                     start=False, stop=False)
                    nc.tensor.matmul(pY, lhsT=ID, rhs=btt1[:, mb, :],
                                     start=False, stop=True)
                    nc.scalar.activation(abBt[:, mb, :], pY, AF.Copy)
                nc.sync.dma_start(
                    out=ab_in[0:HD * HD].rearrange("(kb p n) -> p kb n", p=P, n=HD),
                    in_=abA)
                nc.sync.dma_start(
                    out=ab_in[HD * HD:AB_SZ].rearrange("(kb p n) -> p kb n", p=P, n=D),
                    in_=abBt)

        # ---- AG2 (async; overlapped by the whole attention phase) ----
        nc.gpsimd.collective_compute(
            "AllGather", OP.bypass,
            replica_groups=rgroups,
            ins=[ab_in.opt()], outs=[ab_out.opt()],
        )

        # ============ attention phase (chain interleaved) ============
        nc.vector.memset(RSEL, 0.0)
        with tc.tile_pool(name="patt", bufs=3) as patt, \
             tc.tile_pool(name="pqk", bufs=2) as pqk, \
             tc.tile_pool(name="pwt", bufs=16) as pwt, \
             tc.tile_pool(name="pch", bufs=2) as pch, \
             tc.tile_pool(name="psp", bufs=3, space="PSUM") as psp, \
             tc.tile_pool(name="pssc", bufs=2, space="PSUM") as pssc, \
             tc.tile_pool(name="psat", bufs=2, space="PSUM") as psat:

            def att_block(i, j, b):
                qh = pqk.tile([P, NB, SEG], bf, tag="qh")
                for mb in range(NB):
                    pq = psp.tile([P, SEG], f32, tag="pp")
                    for kb in range(NB):
                        nc.tensor.matmul(
                            pq, lhsT=WQ[:, kb, mb * P:(mb + 1) * P],
                            rhs=XT[i][:, kb, :],
                            start=(kb == 0), stop=(kb == NB - 1))
                    nc.scalar.activation(qh[:, mb, :], pq, AF.Copy)
                    em = patt.tile([P, SEG], bf, tag="em")
                    nc.vector.tensor_scalar_min(em, pq, 0.0)
                    ee = patt.tile([P, SEG], bf, tag="ee")
                    nc.scalar.activation(ee, em, AF.Exp)
                    nc.vector.scalar_tensor_tensor(
                        out=SQ[i][:, mb, :], in0=pq, scalar=1.0, in1=ee,
                        op0=OP.add, op1=OP.max)
                kh = pqk.tile([P, NB, SEG], bf, tag="kh")
                for mb in range(NB):
                    pkt = psp.tile([P, SEG], f32, tag="pp")
                    for kb in range(NB):
                        nc.tensor.matmul(
                            pkt, lhsT=WK[:, kb, mb * P:(mb + 1) * P],
                            rhs=XT[i][:, kb, :],
                            start=(kb == 0), stop=(kb == NB - 1))
                    nc.scalar.activation(kh[:, mb, :], pkt, AF.Copy)

                # software-pipelined: scores(hb+1) issued before av(hb)
                def scores(hb):
                    wts = [[None] * NB, [None] * NB]
                    for kb in range(NB):
                        q0 = kb * P
                        qf = SEG - q0
                        for hh in range(2):
                            ho = hh * 64
                            ps_ = pssc.tile([P, SEG], f32, tag="sc")
                            nc.tensor.matmul(
                                ps_[:, 0:qf],
                                lhsT=kh[ho:ho + 64, hb, q0:q0 + P],
                                rhs=qh[ho:ho + 64, hb, q0:SEG],
                                start=True, stop=True)
                            wt = pwt.tile([P, SEG], bf, tag="wt")
                            nc.scalar.activation(wt[:, 0:qf], ps_[:, 0:qf],
                                                 AF.Exp, scale=0.125)
                            nc.vector.tensor_mul(wt[:, 0:P], wt[:, 0:P], CM)
                            wts[hh][kb] = wt
                    return wts

                def av(hb, wts):
                    for hh in range(2):
                        h = 2 * hb + hh
                        for sb in range(NB):
                            pat = psat.tile([P, D + 1], f32, tag="pat")
                            for kb in range(sb + 1):
                                nc.tensor.matmul(
                                    pat,
                                    lhsT=wts[hh][kb][:, (sb - kb) * P:(sb - kb + 1) * P],
                                    rhs=VAP[i][:, kb, h, :],
                                    start=(kb == 0), stop=(kb == sb))
                            rc = patt.tile([P, 1], f32, tag="rc")
                            nc.vector.reciprocal(rc, pat[:, D:D + 1])
                            if h == 0:
                                nc.vector.tensor_scalar_mul(
                                    ACC[i][:, sb, :], pat[:, 0:D], rc)
                            else:
                                nc.vector.scalar_tensor_tensor(
                                    out=ACC[i][:, sb, :], in0=pat[:, 0:D],
                                    scalar=rc, in1=ACC[i][:, sb, :],
                                    op0=OP.mult, op1=OP.add)

                wts_prev = scores(0)
                for hb in range(1, NB):
                    wts_cur = scores(hb)
                    av(hb - 1, wts_prev)
                    wts_prev = wts_cur
                av(NB - 1, wts_prev)

            def chain_step(step, psch, rprev):
                base = step * AB_SZ
                cA = pch.tile([P, NB, HD], bf, tag="cA")
                cBt = pch.tile([P, NB, D], bf, tag="cBt")
                nc.sync.dma_start(
                    out=cA,
                    in_=ab_out[base:base + HD * HD].rearrange(
                        "(kb p n) -> p kb n", p=P, n=HD))
                nc.sync.dma_start(
                    out=cBt,
                    in_=ab_out[base + HD * HD:base + AB_SZ].rearrange(
                        "(kb p n) -> p kb n", p=P, n=D))
                pRt = psch.tile([P, NB, D], f32, tag="ch")
                for mb in range(NB):
                    if step == 0:
                        nc.tensor.matmul(pRt[:, mb, :], lhsT=ID, rhs=cBt[:, mb, :],
                                         start=True, stop=True)
                    else:
                        for kb in range(NB):
                            nc.tensor.matmul(
                                pRt[:, mb, :], lhsT=cA[:, kb, mb * P:(mb + 1) * P],
                                rhs=rprev[:, kb, :], start=(kb == 0), stop=False)
                        nc.tensor.matmul(pRt[:, mb, :], lhsT=ID,
                                         rhs=rprev[:, mb, :],
                                         start=False, stop=False)
                        nc.tensor.matmul(pRt[:, mb, :], lhsT=ID, rhs=cBt[:, mb, :],
                                         start=False, stop=True)
                rcur = pch.tile([P, NB, D], bf, tag="rcur")
                nc.scalar.activation(rcur, pRt, AF.Copy)
                nc.vector.scalar_tensor_tensor(
                    out=RSEL, in0=rcur, scalar=OH[:, step:step + 1], in1=RSEL,
                    op0=OP.mult, op1=OP.add)
                return rcur

            with tc.tile_pool(name="psch", bufs=1, space="PSUM") as psch:
                rprev = None
                for idx, (j, b) in enumerate(order):
                    att_block(bs_of(b, j), j, b)
                    # chain steps sprinkled after attention blocks 3..6
                    if idx >= 3 and rprev is None:
                        rprev = chain_step(0, psch, None)
                        rprev = chain_step(1, psch, rprev)
                    elif rprev is not None and idx <= 6:
                        rprev = chain_step(idx - 2, psch, rprev)
                for step in range(5, NC - 1):
                    rprev = chain_step(step, psch, rprev)

                # R at segment 2c+1: RLOC1 = a0 RSEL + RSEL + BTT0
                for mb in range(NB):
                    pRl = psat.tile([P, D + 1], f32, tag="pat")
                    for kb in range(NB):
                        nc.tensor.matmul(
                            pRl[:, 0:D], lhsT=AT0[:, kb, mb * P:(mb + 1) * P],
                            rhs=RSEL[:, kb, :], start=(kb == 0), stop=False)
                    nc.tensor.matmul(pRl[:, 0:D], lhsT=ID, rhs=RSEL[:, mb, :],
                                     start=False, stop=False)
                    nc.tensor.matmul(pRl[:, 0:D], lhsT=ID, rhs=BTT0[:, mb, :],
                                     start=False, stop=True)
                    nc.scalar.activation(RLOC1[:, mb, :], pRl[:, 0:D], AF.Copy)

        # ============ phase B: mem path + combine + store ============
        with tc.tile_pool(name="pb", bufs=3) as pb, \
             tc.tile_pool(name="pso", bufs=2, space="PSUM") as pso, \
             tc.tile_pool(name="psd", bufs=2, space="PSUM") as psd:
            for j in range(SPC):
                Rt = RSEL if j == 0 else RLOC1
                for b in range(B):
                    i = bs_of(b, j)
                    for sb in range(NB):
                        po = pso.tile([P, D], f32, tag="po")
                        pd = psd.tile([P, 1], f32, tag="pd")
                        for kb in range(NB):
                            lhsT = SQ[i][:, kb, sb * P:(sb + 1) * P]
                            nc.tensor.matmul(
                                po, lhsT=lhsT, rhs=Rt[:, kb, :],
                                start=(kb == 0), stop=(kb == NB - 1))
                            nc.tensor.matmul(
                                pd, lhsT=lhsT, rhs=ZCOL[:, kb, i:i + 1],
                                start=(kb == 0), stop=(kb == NB - 1))
                        rcm = pb.tile([P, 1], f32, tag="rcm")
                        nc.vector.reciprocal(rcm, pd)
                        ob = pb.tile([P, D], f32, tag="ob")
                        nc.vector.scalar_tensor_tensor(
                            out=ob, in0=po, scalar=rcm, in1=ACC[i][:, sb, :],
                            op0=OP.mult, op1=OP.add)
                        nc.sync.dma_start(
                            out=out_d[b, j, sb * P:(sb + 1) * P, :], in_=ob)

    nc.compile()
    return nc


def _prep_inputs(x, Wq, Wk, Wv, Wd, beta):
    """Host-side prep: fold gates/Wd into Wv, transpose/cast/shard."""
    g = 1.0 / (1.0 + np.exp(-beta.astype(np.float64)))  # (H,)
    g = g.astype(np.float32)
    grep = np.repeat(g, D)

    # Wv' : attention value proj with (1-g_h) Wd[h-block] folded per head
    Wvp = np.zeros((DIN, HD), np.float32)
    for h in range(H):
        Wvp[:, h * D:(h + 1) * D] = (
            Wv[:, h * D:(h + 1) * D] @ ((1.0 - g[h]) * Wd[h * D:(h + 1) * D]))
    # Wvt : vtil proj  (v @ diag(g) Wd  folded)
    Wvt = (Wv * grep[None, :]) @ Wd  # (DIN, D)

    def wprep(w):
        return np.ascontiguousarray(w.reshape(NB, P, w.shape[1]).astype(bf_np))

    wq_a, wk_a = wprep(Wq), wprep(Wk)
    wvp_a, wvt_a = wprep(Wvp), wprep(Wvt)
    cmask = np.triu(np.ones((P, P), np.float32)).astype(bf_np)
    ident = np.eye(P, dtype=np.float32).astype(bf_np)

    xs = x.reshape(B, NSEG, SEG, DIN)
    in_maps = []
    for c in range(NC):
        xloc = xs[:, 2 * c:2 * c + 2]                        # (B, SPC, SEG, DIN)
        xt = xloc.transpose(0, 1, 3, 2)                      # (B, SPC, DIN, SEG)
        xt = np.ascontiguousarray(
            xt.reshape(B, SPC, NB, P, SEG).astype(bf_np))
        # AG1 global row for (t, b): rank t//2 contributes row (t%2)*B + b
        zmask = np.zeros((64, NC), np.float32)
        for jj in range(NC):
            tgt = 2 * c + (jj // B)
            bb = jj % B
            for t in range(NSEG):
                if t < tgt:
                    zmask[(t // 2) * BS + (t % 2) * B + bb, jj] = 1.0
        oh = np.zeros((P, NC), np.float32)
        if c >= 1:
            oh[:, c - 1] = 1.0
        in_maps.append({
            "xt": xt, "wq": wq_a, "wk": wk_a, "wvp": wvp_a, "wvt": wvt_a,
            "zmask": zmask, "oh": oh, "cmask": cmask, "ident": ident,
        })
    return in_maps


def kernel(x, Wq, Wk, Wv, Wd, beta, _trace=False):
    x = np.asarray(x, np.float32)
    in_maps = _prep_inputs(
        x, np.asarray(Wq, np.float32), np.asarray(Wk, np.float32),
        np.asarray(Wv, np.float32), np.asarray(Wd, np.float32),
        np.asarray(beta, np.float32))
    if "nc" not in _CACHE:
        _CACHE["nc"] = _build()
    nc = _CACHE["nc"]
    res = bass_utils.run_bass_kernel_spmd(
        nc, in_maps, core_ids=list(range(NC)), trace=_trace)
    _CACHE["last_results"] = res
    out = np.empty((B, L, D), np.float32)
    for c in range(NC):
        oc = res.results[c]["out"]                  # (B, SPC, SEG, D)
        out[:, 2 * c * SEG:(2 * c + 2) * SEG, :] = oc.reshape(B, SPC * SEG, D)
    return out
